# revision 4
# baseline (speedup 1.0000x reference)
"""MK-MMD loss kernel for Trainium2 (8 NeuronCores, data-parallel).

Reference computation:
    pairs (x,x',y,y') from consecutive rows of Xs/Xt
    D1=|x-x'|^2, D2=|y-y'|^2, D3=|x-y'|^2, D4=|x'-y|^2
    h_u[g,p] = sum_c sign(c) * exp(-D[p,c] / (2*gamma_g^2)),  29 gammas
    out = betas^T @ mean_p(h_u)          -> shape (1,)

Strategy (memory-bound: 512MB of X data streamed once):
  - Shard samples across 8 cores (16384 rows of each of Xs/Xt per core).
  - Expand |a-b|^2 = sum a^2 + sum b^2 - 2 sum ab so the inner loop needs no
    subtract/square/reduce passes:
      * 4 norms/pair  -> ScalarE  activation(Square, accum_out=...)  (fused)
      * 4 dots/pair   -> VectorE  tensor_tensor_reduce(mult, add)    (fused)
    which splits the elementwise work across both engines, each running
    below the DMA streaming time per tile.
  - Per-pair D values land in small column buffers; a tail stage computes
    exp for the 29 length-scales (ACT) and signed sums (DVE), emitting a
    [128, 29] per-partition partial result per core.
  - Host: sum partials over partitions/cores, divide by pair count, dot betas.
"""

import numpy as np

import concourse.bacc as bacc
import concourse.tile as tile
from concourse import mybir
from concourse.bass_utils import run_bass_kernel_spmd

N_SAMPLES = 131072
N_FEAT = 512
N_KERNELS = 29
N_CORES = 8

ROWS_PER_CORE = N_SAMPLES // N_CORES        # 16384 samples per core
PAIRS_PER_CORE = ROWS_PER_CORE // 2         # 8192
# View each core's [16384, 512] shard as [4096, 2048]: one row = 2 pairs,
# each pair = 1024 contiguous floats. A [128, 2048] tile = 256 pairs,
# fully contiguous 1MB DMA.
QROWS = ROWS_PER_CORE // 4                  # 4096
N_ITERS = QROWS // 128                      # 32 tiles per core
N_COLS = 2 * N_ITERS                        # 64 accumulator columns

F32 = mybir.dt.float32
ALU = mybir.AluOpType
ACTF = mybir.ActivationFunctionType


def _gamma_scales() -> np.ndarray:
    gammas = np.power(
        np.float32(2.0), np.arange(-3.5, 3.75, 0.25, dtype=np.float32)
    ).astype(np.float32)
    return (1.0 / (2.0 * gammas * gammas)).astype(np.float32)


def _build():
    nc = bacc.Bacc("TRN2", target_bir_lowering=False, debug=False)

    xs_d = nc.dram_tensor("Xs", [QROWS, 2048], F32, kind="ExternalInput").ap()
    xt_d = nc.dram_tensor("Xt", [QROWS, 2048], F32, kind="ExternalInput").ap()
    r_d = nc.dram_tensor("R", [128, N_KERNELS], F32, kind="ExternalOutput").ap()

    inv2g2 = _gamma_scales()

    with tile.TileContext(nc) as tc:
        with (
            tc.tile_pool(name="xin", bufs=3) as xin_pool,
            tc.tile_pool(name="acc", bufs=1) as acc_pool,
            tc.tile_pool(name="scratch", bufs=1) as scratch_pool,
            tc.tile_pool(name="ebuf", bufs=2) as ebuf_pool,
        ):
            # Per-pair accumulator columns: one [128,1] column per 128 pairs.
            NA = acc_pool.tile([128, N_COLS], F32)   # |x|^2
            NB = acc_pool.tile([128, N_COLS], F32)   # |x'|^2
            NC = acc_pool.tile([128, N_COLS], F32)   # |y|^2
            ND = acc_pool.tile([128, N_COLS], F32)   # |y'|^2
            AB = acc_pool.tile([128, N_COLS], F32)   # x.x'
            CD = acc_pool.tile([128, N_COLS], F32)   # y.y'
            AD = acc_pool.tile([128, N_COLS], F32)   # x.y'
            BC = acc_pool.tile([128, N_COLS], F32)   # x'.y

            dve_dump = scratch_pool.tile([128, N_FEAT], F32)
            act_dump = scratch_pool.tile([128, N_FEAT], F32)

            for t in range(N_ITERS):
                xs = xin_pool.tile([128, 2048], F32, tag="xs")
                nc.default_dma_engine.dma_start(
                    out=xs[:], in_=xs_d[t * 128 : (t + 1) * 128, :]
                )
                xt = xin_pool.tile([128, 2048], F32, tag="xt")
                nc.default_dma_engine.dma_start(
                    out=xt[:], in_=xt_d[t * 128 : (t + 1) * 128, :]
                )
                for sp in range(2):
                    c = 2 * t + sp
                    o = sp * 1024
                    a = xs[:, o : o + 512]
                    b = xs[:, o + 512 : o + 1024]
                    yc = xt[:, o : o + 512]
                    yd = xt[:, o + 512 : o + 1024]

                    # VectorE: fused elementwise-mult + free-dim reduce.
                    for (i0, i1, dst) in (
                        (a, b, AB),
                        (yc, yd, CD),
                        (a, yd, AD),
                        (b, yc, BC),
                    ):
                        nc.vector.scalar_tensor_tensor(
                            out=dve_dump[:],
                            in0=i0,
                            scalar=1.0,
                            in1=i1,
                            op0=ALU.bypass,
                            op1=ALU.mult,
                            accum_out=dst[:, c : c + 1],
                        )
                    # ScalarE: fused square + free-dim accumulate.
                    for (src, dst) in ((a, NA), (b, NB), (yc, NC), (yd, ND)):
                        nc.scalar.activation(
                            act_dump[:],
                            src,
                            ACTF.Square,
                            accum_out=dst[:, c : c + 1],
                        )

            # D assembly: D = (norm_i + norm_j) - 2*dot  -> Dbuf [128, 256]
            # cols [0:64]=D1 [64:128]=D2 (positive), [128:192]=D3 [192:256]=D4.
            dbuf = acc_pool.tile([128, 4 * N_COLS], F32)
            tsum = scratch_pool.tile([128, N_COLS], F32)
            for d, (u, v, w) in enumerate(
                ((NA, NB, AB), (NC, ND, CD), (NA, ND, AD), (NB, NC, BC))
            ):
                nc.vector.tensor_add(tsum[:], u[:], v[:])
                nc.vector.scalar_tensor_tensor(
                    out=dbuf[:, d * N_COLS : (d + 1) * N_COLS],
                    in0=w[:],
                    scalar=-2.0,
                    in1=tsum[:],
                    op0=ALU.mult,
                    op1=ALU.add,
                )

            # Gamma stage: for each length-scale, E = exp(-s*D) on ACT, then
            # signed sum (pos block - neg block, reduced over pairs) on DVE.
            rbuf = acc_pool.tile([128, N_KERNELS], F32)
            gdump = scratch_pool.tile([128, 2 * N_COLS], F32)
            for g in range(N_KERNELS):
                ebuf = ebuf_pool.tile([128, 4 * N_COLS], F32, tag="ebuf")
                nc.scalar.activation(
                    ebuf[:], dbuf[:], ACTF.Exp, scale=float(-inv2g2[g])
                )
                nc.vector.scalar_tensor_tensor(
                    out=gdump[:],
                    in0=ebuf[:, 0 : 2 * N_COLS],
                    scalar=1.0,
                    in1=ebuf[:, 2 * N_COLS : 4 * N_COLS],
                    op0=ALU.bypass,
                    op1=ALU.subtract,
                    accum_out=rbuf[:, g : g + 1],
                )

            nc.default_dma_engine.dma_start(out=r_d[:, :], in_=rbuf[:])

    nc.compile()
    return nc


_NC_CACHE = None


def _get_nc():
    global _NC_CACHE
    if _NC_CACHE is None:
        _NC_CACHE = _build()
    return _NC_CACHE


def _make_in_maps(Xs: np.ndarray, Xt: np.ndarray):
    in_maps = []
    for i in range(N_CORES):
        sl = slice(i * ROWS_PER_CORE, (i + 1) * ROWS_PER_CORE)
        in_maps.append(
            {
                "Xs": np.ascontiguousarray(Xs[sl]).reshape(QROWS, 2048),
                "Xt": np.ascontiguousarray(Xt[sl]).reshape(QROWS, 2048),
            }
        )
    return in_maps


def _finish(results, betas: np.ndarray) -> np.ndarray:
    tot = np.zeros(N_KERNELS, dtype=np.float64)
    for r in results:
        tot += r["R"].astype(np.float64).sum(axis=0)
    hat_d = tot / (N_SAMPLES // 2)
    out = betas.astype(np.float64).reshape(N_KERNELS) @ hat_d
    return np.array([out], dtype=np.float32)


def run(Xs, Xt, betas, **spmd_kwargs):
    """Run the device kernel; returns (output, BassKernelResults)."""
    nc = _get_nc()
    in_maps = _make_in_maps(np.asarray(Xs), np.asarray(Xt))
    res = run_bass_kernel_spmd(nc, in_maps, list(range(N_CORES)), **spmd_kwargs)
    return _finish(res.results, np.asarray(betas)), res


def kernel(Xs, Xt, betas):
    out, _ = run(Xs, Xt, betas)
    return out


# revision 5
# speedup vs baseline: 1.2064x; 1.2064x over previous
"""MK-MMD loss kernel for Trainium2 (8 NeuronCores, data-parallel).

Reference computation:
    pairs (x,x',y,y') from consecutive rows of Xs/Xt
    D1=|x-x'|^2, D2=|y-y'|^2, D3=|x-y'|^2, D4=|x'-y|^2
    h_u[g,p] = exp(-D1*s_g) + exp(-D2*s_g) - exp(-D3*s_g) - exp(-D4*s_g)
    out = betas^T @ mean_p(h_u)          -> shape (1,)

Strategy (memory-bound: 512MB of X data streamed once):
  - Shard samples across 8 cores (16384 rows of each of Xs/Xt per core).
  - A custom DVE ucode op SQDIFF_REDUCE computes out=(a-b)^2 with a fused
    free-dim sum (accum_out), so each of the four pair distances is ONE
    VectorE instruction per 128 pairs. No norms/dots/fixups; ScalarE only
    runs the 29 exp ops of the gamma stage at the end.
  - Per-pair D values land in a [128, 256] column buffer; the tail stage
    computes exp(-s_g * D) (ACT, one op per gamma) and signed sums
    (VectorE scalar_tensor_tensor subtract + accum), emitting [128, 29]
    per-partition partials per core.
  - Host: sum partials over partitions/cores, divide by pair count, dot
    betas.
"""

import numpy as np

import concourse.bacc as bacc
import concourse.tile as tile
from concourse import dve_ops, mybir
from concourse.bass_utils import run_bass_kernel_spmd
from concourse.dve_spec import Spec, Src0, Src1, Zero, _has_src1, lower, sq
from concourse.dve_uop import DveOpSpec
from operator import add as _operator_add

N_SAMPLES = 131072
N_FEAT = 512
N_KERNELS = 29
N_CORES = 8

ROWS_PER_CORE = N_SAMPLES // N_CORES        # 16384 samples per core
PAIRS_PER_CORE = ROWS_PER_CORE // 2         # 8192
# View each core's [16384, 512] shard as [4096, 2048]: one row = 2 pairs,
# each pair = 1024 contiguous floats. A [128, 2048] tile = 256 pairs,
# fully contiguous 1MB DMA.
QROWS = ROWS_PER_CORE // 4                  # 4096
N_ITERS = QROWS // 128                      # 32 tiles per core
N_COLS = 2 * N_ITERS                        # 64 accumulator columns

F32 = mybir.dt.float32
ALU = mybir.AluOpType
ACTF = mybir.ActivationFunctionType


def _make_sqdiff_op():
    """Register the SQDIFF_REDUCE custom DVE op: out=(in0-in1)^2,
    accum_out=sum(out) along the free dim. Idempotent."""
    for op in dve_ops.OPS:
        if op.name == "SQDIFF_REDUCE":
            return op

    def _ref(in0, in1, c0, c1, c2):
        b = ((in0.astype(np.float32) - in1) ** 2).astype(np.float32)
        return b, b.reshape(b.shape[0], -1).sum(axis=-1, keepdims=True)

    spec = Spec(
        body=sq(Src0 - Src1), accum=_operator_add, accum_init=Zero, reference=_ref
    )
    opcode = max(dve_ops._SUB_OPCODE_FOR_NAME.values()) + 1
    assert opcode < 0x20, "custom DVE opcode rows exhausted"
    dve_ops._SUB_OPCODE_FOR_NAME["SQDIFF_REDUCE"] = opcode
    shas = {
        ver: DveOpSpec(
            name="SQDIFF_REDUCE",
            opcode=opcode,
            uops=lower(spec, ver=ver),
            rd1_en=_has_src1(spec),
        ).sha(ver)
        for ver in ("v3", "v4")
    }
    op = dve_ops.DveOp("SQDIFF_REDUCE", spec, subdim=False, uops_sha=shas)
    dve_ops.OPS.append(op)
    dve_ops.CUSTOM_DVE_SPECS[op.name] = op.spec
    return op


SQDIFF = _make_sqdiff_op()


def _gamma_scales() -> np.ndarray:
    gammas = np.power(
        np.float32(2.0), np.arange(-3.5, 3.75, 0.25, dtype=np.float32)
    ).astype(np.float32)
    return (1.0 / (2.0 * gammas * gammas)).astype(np.float32)


def _build():
    nc = bacc.Bacc("TRN2", target_bir_lowering=False, debug=False)

    xs_d = nc.dram_tensor("Xs", [QROWS, 2048], F32, kind="ExternalInput").ap()
    xt_d = nc.dram_tensor("Xt", [QROWS, 2048], F32, kind="ExternalInput").ap()
    r_d = nc.dram_tensor("R", [128, N_KERNELS], F32, kind="ExternalOutput").ap()

    inv2g2 = _gamma_scales()

    with tile.TileContext(nc) as tc:
        with (
            tc.tile_pool(name="xin", bufs=3) as xin_pool,
            tc.tile_pool(name="acc", bufs=1) as acc_pool,
            tc.tile_pool(name="scratch", bufs=1) as scratch_pool,
            tc.tile_pool(name="ebuf", bufs=2) as ebuf_pool,
        ):
            # D columns: [0:64]=D1 [64:128]=D2 (positive), [128:192]=D3
            # [192:256]=D4 (negative).
            dbuf = acc_pool.tile([128, 4 * N_COLS], F32)
            dve_dump = scratch_pool.tile([128, N_FEAT], F32)

            for t in range(N_ITERS):
                xs = xin_pool.tile([128, 2048], F32, tag="xs")
                nc.default_dma_engine.dma_start(
                    out=xs[:], in_=xs_d[t * 128 : (t + 1) * 128, :]
                )
                xt = xin_pool.tile([128, 2048], F32, tag="xt")
                nc.default_dma_engine.dma_start(
                    out=xt[:], in_=xt_d[t * 128 : (t + 1) * 128, :]
                )
                for sp in range(2):
                    c = 2 * t + sp
                    o = sp * 1024
                    a = xs[:, o : o + 512]
                    b = xs[:, o + 512 : o + 1024]
                    yc = xt[:, o : o + 512]
                    yd = xt[:, o + 512 : o + 1024]
                    for d, (i0, i1) in enumerate(
                        ((a, b), (yc, yd), (a, yd), (b, yc))
                    ):
                        nc.vector._custom_dve(
                            SQDIFF,
                            out=dve_dump[:],
                            accum_out=dbuf[:, d * N_COLS + c : d * N_COLS + c + 1],
                            in0=i0,
                            in1=i1,
                        )

            # Gamma stage: for each length-scale, E = exp(-s*D) on ACT, then
            # signed sum (pos block - neg block, reduced over pairs) on DVE.
            rbuf = acc_pool.tile([128, N_KERNELS], F32)
            gdump = scratch_pool.tile([128, 2 * N_COLS], F32)
            for g in range(N_KERNELS):
                ebuf = ebuf_pool.tile([128, 4 * N_COLS], F32, tag="ebuf")
                nc.scalar.activation(
                    ebuf[:], dbuf[:], ACTF.Exp, scale=float(-inv2g2[g])
                )
                nc.vector.scalar_tensor_tensor(
                    out=gdump[:],
                    in0=ebuf[:, 0 : 2 * N_COLS],
                    scalar=1.0,
                    in1=ebuf[:, 2 * N_COLS : 4 * N_COLS],
                    op0=ALU.bypass,
                    op1=ALU.subtract,
                    accum_out=rbuf[:, g : g + 1],
                )

            nc.default_dma_engine.dma_start(out=r_d[:, :], in_=rbuf[:])

    nc.compile()
    return nc


_NC_CACHE = None


def _get_nc():
    global _NC_CACHE
    if _NC_CACHE is None:
        _NC_CACHE = _build()
    return _NC_CACHE


def _make_in_maps(Xs: np.ndarray, Xt: np.ndarray):
    in_maps = []
    for i in range(N_CORES):
        sl = slice(i * ROWS_PER_CORE, (i + 1) * ROWS_PER_CORE)
        in_maps.append(
            {
                "Xs": np.ascontiguousarray(Xs[sl]).reshape(QROWS, 2048),
                "Xt": np.ascontiguousarray(Xt[sl]).reshape(QROWS, 2048),
            }
        )
    return in_maps


def _finish(results, betas: np.ndarray) -> np.ndarray:
    tot = np.zeros(N_KERNELS, dtype=np.float64)
    for r in results:
        tot += r["R"].astype(np.float64).sum(axis=0)
    hat_d = tot / (N_SAMPLES // 2)
    out = betas.astype(np.float64).reshape(N_KERNELS) @ hat_d
    return np.array([out], dtype=np.float32)


def run(Xs, Xt, betas, **spmd_kwargs):
    """Run the device kernel; returns (output, BassKernelResults)."""
    nc = _get_nc()
    in_maps = _make_in_maps(np.asarray(Xs), np.asarray(Xt))
    res = run_bass_kernel_spmd(nc, in_maps, list(range(N_CORES)), **spmd_kwargs)
    return _finish(res.results, np.asarray(betas)), res


def kernel(Xs, Xt, betas):
    out, _ = run(Xs, Xt, betas)
    return out
